# revision 6
# baseline (speedup 1.0000x reference)
"""Analytic Gaussian VP score on 8 TRN2 NeuronCores.

Math: per sample i, score_i = -Sigma_i^{-1} (x_i - a_i*mean0) with
Sigma_i = a_i^2*cov0 + s_i^2*I.  All Sigma_i are shifted/scaled versions of
one shared matrix, so instead of 128 per-sample Choleskys we apply a
per-sample degree-NK Chebyshev polynomial of cov0:

    score_i = -sum_k c_{i,k} T_k(Mt) u_i,   Mt = (cov0 - MID*I)/HALF

The coefficients c_{i,k} are computed ON DEVICE from t_i (ScalarE exp +
DVE + small PE matmuls).  T_k are generated four at a time via
T4h = 2*gamma^4*T_4(Mt) built from two matrix squarings; the identity
T_{k+4} = 2*T4*T_k - T_{k-4} advances four chains per batched matmul so
the TensorEngine runs [128,128]-weight matmuls with a 64-wide moving
operand.  Basis scaling gamma = HALF/2 folds all affine constants into
shifted matrices: Cs = C - MID*I (diag-block fixups only), Bs = Cs**2
- 2g^2*I, T4h = Bs**2 - 2g^4*I, with Y1 = 0.5*Cs@Y0, Y2 = 0.5*Bs@Y0,
Y3 = Bs@Y1 - g^2*Y1 - so no full-matrix elementwise passes exist
between the squarings, only [128,128] diag ops and the PSUM->SBUF
moves that are needed anyway.

Perf structure (from perfetto/NTFF analysis of the previous version):
- every HWDGE dma_start costs ~600-790ns of issue time on its engine, so
  all small inputs (x, mean, t, eye, lam, dmat) are host-packed into ONE
  [128, 229] tensor; cov0 ships as 4 chunk DMAs split over both HWDGE
  queues; the ones-vectors are gpsimd memsets (6 input DMAs, was 14).
- C^2 runs kc-major (each cov chunk's 4 matmuls fire on chunk arrival,
  accumulating into 4 PSUM banks) instead of waiting for the full 1MB.
- the PE stream (warmup -> Cs^2 -> p1 -> Bs^2(+p2 riding) -> chain
  steps) has no >3.4us gaps, so the HAM clock gate releases once at
  ~10us and stays at 2.4GHz (the old version re-throttled mid-kernel).
- NK=15 (16 coeffs = 4 chains x 4 steps): pole-model error ~9.5e-3 vs
  the 2e-2 gate (measured 2.7e-3 at NK=19, model-accurate).
- ~7us of exec window (runtime preamble + walrus' 253-semaphore clear
  epilogue + final barriers) is fixed overhead outside kernel control.

Sharding: pure data parallelism - mean0/cov0 replicated, the 128 (t, x)
pairs split 16 per core, no collectives.  State kept transposed
([feature, (chunk, chain, sample)]) so matmuls are cov0-stationary
(symmetric blocks, no transposes).  float32r on construction matmuls,
bf16 hi+lo split of T4h (~2^-16 effective weights) on chain steps.
"""

import numpy as np

try:
    import concourse.bass as bass
except ImportError:  # fresh grading dir: point at the staged repo
    import sys

    for _p in ("/opt/trn_rl_repo", "/root/.axon_site/_ro/trn_rl_repo"):
        if _p not in sys.path:
            sys.path.insert(0, _p)
    import concourse.bass as bass

from contextlib import ExitStack

import concourse.tile as tile
from concourse import bacc, mybir
from concourse.tile import ScopedClock


def _lean_drain_and_barrier(self, tick_clock, wait_clock):
    """Tile end-sequence without the TileContext per-semaphore end-clears.

    Bass clears the kernel semaphore range at NEFF entry, so for a
    single-TileContext kernel the end-of-kernel clear_and_free pass is
    redundant.  Keep the drain (fences DMA queues) and one barrier.
    """
    drain_inst = self.nc.sync.drain()
    wait_clock.add_sem_waits(
        drain_inst.ins, ScopedClock({None: tick_clock.global_clock})
    )
    self.nc.all_engine_barrier()
    popped = self.nc._tile_sem_poison_stack.pop()
    assert popped is self._sem_poison
from concourse.bass_utils import run_bass_kernel_spmd

F32 = mybir.dt.float32
F32R = mybir.dt.float32r
BF16 = mybir.dt.bfloat16
AL = mybir.AluOpType
AX = mybir.AxisListType

B, D = 128, 512
NCORES = 8
BLOC = B // NCORES  # 16 samples per core
KC = D // 128  # 4 partition chunks of the feature dim
NCH = 4  # Chebyshev chains advanced per step
W = NCH * BLOC  # 64: moving-operand width of the chain matmuls

# Chebyshev setup (input-independent constants)
L_BND, U_BND = 0.0995, 4.10
NN = 64  # interpolation nodes
NK = 15  # polynomial degree; NK+1 = 16 coefficients = 4 chains x 4 steps
NSTEP = (NK + 1) // 4 - 1  # chain steps after init (first one special)
MID = (U_BND + L_BND) / 2.0
HALF = (U_BND - L_BND) / 2.0
GAMMA = HALF / 2.0  # basis scaling; matmuls consume raw shifted matrices
G2 = GAMMA * GAMMA
G8 = GAMMA**8
TWO_G2 = 2.0 * G2  # Bs = Cs^2 - 2g^2 I
TWO_G4 = 2.0 * GAMMA**4  # T4h = Bs^2 - 2g^4 I
BETA_MIN, BETA_MAX = 0.1, 20.0

# xpack column layout: [xhat 64 | mhat 4 | trow 16 (row 0) | eye 128 |
#                       lam 1 (rows 0:NN) | dmat NK+1 (rows 0:NN)]
XP_X = 0
XP_M = XP_X + KC * BLOC
XP_T = XP_M + KC
XP_EYE = XP_T + BLOC
XP_LAM = XP_EYE + 128
XP_DM = XP_LAM + 1
XP_W = XP_DM + (NK + 1)


def _host_constants():
    j = np.arange(NN)
    th = np.pi * (j + 0.5) / NN
    lam = (MID + HALF * np.cos(th)).astype(np.float32).reshape(NN, 1)
    k = np.arange(NK + 1)
    dm = (2.0 / NN) * np.cos(k[None, :] * th[:, None])
    dm[:, 0] *= 0.5
    dm = (-dm) * (1.0 / np.float64(GAMMA)) ** k[None, :]  # fold -1, gamma^-k
    dmat = dm.astype(np.float32)
    return lam, dmat


def _build_nc():
    nc = bacc.Bacc()
    xpack = nc.declare_dram_parameter("xpack", [128, XP_W], F32, isOutput=False)
    cov0 = nc.declare_dram_parameter("cov0", [D, D], F32R, isOutput=False)
    outT = nc.declare_dram_parameter("outT", [D, BLOC], F32, isOutput=True)

    with ExitStack() as ctx:
        tc = ctx.enter_context(tile.TileContext(nc))
        tc._drain_and_barrier = _lean_drain_and_barrier.__get__(tc)
        const = ctx.enter_context(tc.tile_pool(name="const", bufs=1))
        state = ctx.enter_context(tc.tile_pool(name="state", bufs=1))
        work = ctx.enter_context(tc.tile_pool(name="work", bufs=2))
        psq = ctx.enter_context(tc.tile_pool(name="psq", bufs=4, space="PSUM"))
        ps_stp = ctx.enter_context(tc.tile_pool(name="ps_stp", bufs=1, space="PSUM"))
        ps_mv = ctx.enter_context(tc.tile_pool(name="ps_mv", bufs=1, space="PSUM"))
        ps_one = ctx.enter_context(tc.tile_pool(name="ps_one", bufs=1, space="PSUM"))

        # ---- PE warm-up: memset a scratch tile (no DMA dependency) + dummy
        # fp32 matmuls so the HAM clock gate sees activity from the start;
        # sized to end roughly when the first cov chunk lands (~1us).
        warm_sb = const.tile([128, 128], F32, tag="warm_sb")
        nc.gpsimd.memset(warm_sb[:], 1.0)
        warm_ps = ps_one.tile([128, (NK + 1) * BLOC], F32, tag="one", name="warm_ps")
        for _ in range(3):
            nc.tensor.matmul(warm_ps[:, 0:128], warm_sb[:], warm_sb[:])

        # ---- loads: 4 cov chunks split over both HWDGE queues + one packed
        # tensor with everything else ----
        cs = [
            const.tile([128, D], F32R, tag=f"cs{k}", name=f"cs{k}")
            for k in range(KC)
        ]
        xp = const.tile([128, XP_W], F32, tag="xp")
        nc.sync.dma_start(cs[0][:], cov0[0:128, :])
        nc.scalar.dma_start(xp[:], xpack[:])
        nc.sync.dma_start(cs[1][:], cov0[128:256, :])
        nc.scalar.dma_start(cs[2][:], cov0[256:384, :])
        nc.sync.dma_start(cs[3][:], cov0[384:512, :])

        eye = xp[:, XP_EYE : XP_EYE + 128]
        xhat = xp[:, XP_X : XP_X + KC * BLOC]
        mhat = xp[:, XP_M : XP_M + KC]
        trow = xp[0:1, XP_T : XP_T + BLOC]
        lam_sb = xp[0:NN, XP_LAM : XP_LAM + 1]
        dmat_sb = xp[0:NN, XP_DM : XP_DM + NK + 1]

        ones1 = const.tile([1, 128], F32, tag="ones1")
        nc.gpsimd.memset(ones1[:], 1.0)
        ones_nn = const.tile([NN, 128], F32, tag="ones_nn")
        nc.gpsimd.memset(ones_nn[:], 1.0)

        # ---- Cs = C - MID*I: in-place diag-block fixups as chunks land ----
        for k in range(KC):
            db = cs[k][:, k * 128 : (k + 1) * 128]
            nc.vector.scalar_tensor_tensor(db, eye, -MID, db, AL.mult, AL.add)

        # ---- Cs^2 kc-major: each chunk's 4 matmuls fire on arrival ----
        c2 = [
            psq.tile([128, D], F32, tag="sq", name=f"c2_{r}")
            for r in range(KC)
        ]
        for kc in range(KC):
            for r in range(KC):
                nc.tensor.matmul(
                    c2[r][:],
                    cs[kc][:, r * 128 : (r + 1) * 128],
                    cs[kc][:],
                    start=(kc == 0),
                    stop=(kc == KC - 1),
                )

        # ---- per-sample scalars from t (overlaps the cov DMAs) ----
        u9 = const.tile([1, BLOC], F32, tag="u9")
        nc.vector.tensor_scalar(u9[:], trow, 9.95, 0.1, AL.mult, AL.add)
        ib = const.tile([1, BLOC], F32, tag="ib")
        nc.vector.tensor_mul(ib[:], u9[:], trow)
        a_row = const.tile([1, BLOC], F32, tag="a_row")
        nc.scalar.activation(
            a_row[:], ib[:], mybir.ActivationFunctionType.Exp, scale=-0.5
        )
        abc = const.tile([1, 3 * BLOC], F32, tag="abc")  # [a | a^2 | s^2]
        nc.vector.tensor_copy(abc[:, 0:BLOC], a_row[:])
        nc.vector.tensor_mul(abc[:, BLOC : 2 * BLOC], a_row[:], a_row[:])
        nc.vector.tensor_scalar(
            abc[:, 2 * BLOC :], abc[:, BLOC : 2 * BLOC], -1.0, 1.0, AL.mult, AL.add
        )
        nc.vector.tensor_scalar_max(abc[:, 2 * BLOC :], abc[:, 2 * BLOC :], 1e-12)

        # broadcast [a | a^2 | s^2] down all 128 partitions via a K=1 matmul
        rep_ps = ps_one.tile([128, (NK + 1) * BLOC], F32, tag="one", name="rep_ps")
        nc.tensor.matmul(rep_ps[:, 0 : 3 * BLOC], ones1[:], abc[:])
        rep = const.tile([128, 3 * BLOC], F32, tag="rep_sb")
        nc.scalar.copy(rep[:], rep_ps[:, 0 : 3 * BLOC])
        a_rep = rep[:, 0:BLOC]
        a2_rep = rep[:, BLOC : 2 * BLOC]
        s2_rep = rep[:, 2 * BLOC : 3 * BLOC]

        # ---- Chebyshev coefficients on device ----
        q = const.tile([NN, BLOC], F32, tag="q")
        nc.vector.scalar_tensor_tensor(
            q[:], a2_rep[0:NN, :], lam_sb[:, 0:1], s2_rep[0:NN, :], AL.mult, AL.add
        )
        fhat = const.tile([NN, BLOC], F32, tag="fhat")
        nc.vector.reciprocal(fhat[:], q[:])
        rhs_t = const.tile([NN, (NK + 1) * BLOC], F32, tag="rhs_t")
        nc.vector.tensor_mul(
            rhs_t[:].rearrange("p (k i) -> p k i", k=NK + 1),
            fhat[:].unsqueeze(1).broadcast_to((NN, NK + 1, BLOC)),
            dmat_sb.unsqueeze(2).broadcast_to((NN, NK + 1, BLOC)),
        )
        c_ps = ps_one.tile([128, (NK + 1) * BLOC], F32, tag="one", name="c_ps")
        nc.tensor.matmul(c_ps[:], ones_nn[:], rhs_t[:])
        c_sb = const.tile([128, (NK + 1) * BLOC], F32, tag="c_sb")
        nc.scalar.copy(c_sb[:], c_ps[:])

        def cstep(s):
            """coefficients for step s: [128, (chain, sample)] bcast over kc."""
            return (
                c_sb[:, s * W : (s + 1) * W]
                .unsqueeze(1)
                .broadcast_to((128, KC, W))
            )

        # ---- state: X [128, (kc, chain, sample)], bf16 ----
        xs = [
            state.tile([128, KC * W], BF16, tag=f"X{i}", name=f"X{i}")
            for i in range(3)
        ]
        y01 = state.tile([128, KC * 2 * BLOC], F32R, tag="y01")  # f32 Y0,Y1
        acc = state.tile([128, KC * W], F32, tag="acc")

        def chain(st, r):
            """[128, kc, BLOC] view of chain r of state tile st."""
            return st[:].rearrange("p (k r i) -> p k r i", k=KC, r=NCH)[:, :, r, :]

        def v3(ap):
            return ap.rearrange("p (k i) -> p k i", k=KC)

        # ---- init: Y0 = u = x - a*mean0 (f32 scratch y01 + bf16 chain) ----
        x0 = xs[0]
        y01v = y01[:].rearrange("p (k r i) -> p k r i", k=KC, r=2)
        w1 = work.tile([128, KC * BLOC], F32, tag="w1")
        nc.vector.tensor_mul(
            v3(w1[:]),
            a_rep.unsqueeze(1).broadcast_to((128, KC, BLOC)),
            mhat.unsqueeze(2).broadcast_to((128, KC, BLOC)),
        )
        nc.vector.tensor_sub(y01v[:, :, 0, :], v3(xhat), v3(w1[:]))
        nc.gpsimd.tensor_copy(chain(x0, 0), y01v[:, :, 0, :])

        # ---- Bs = Cs^2 - 2g^2*I in SBUF fp32r (PSUM->SBUF moves split
        # between ScalarE and DVE; diag fixed in place) ----
        bs = [
            const.tile([128, D], F32R, tag=f"bs{r}", name=f"bs{r}")
            for r in range(KC)
        ]
        for r in range(KC):
            if r < 2:
                nc.scalar.copy(bs[r][:], c2[r][:])
            else:
                nc.vector.tensor_copy(bs[r][:], c2[r][:])
            db = bs[r][:, r * 128 : (r + 1) * 128]
            nc.vector.scalar_tensor_tensor(db, eye, -TWO_G2, db, AL.mult, AL.add)

        # ---- p1: Y1 = 0.5*Cs@Y0 (runs while the Bs copies drain) ----
        p1 = ps_mv.tile([128, KC * BLOC], F32, tag="pmv", name="p1")
        for mc in range(KC):
            for kc in range(KC):
                nc.tensor.matmul(
                    p1[:, mc * BLOC : (mc + 1) * BLOC],
                    cs[kc][:, mc * 128 : (mc + 1) * 128],
                    y01v[:, kc, 0, :].bitcast(F32R),
                    start=(kc == 0),
                    stop=(kc == KC - 1),
                )
        nc.vector.tensor_scalar_mul(y01v[:, :, 1, :], v3(p1[:]), 0.5)
        nc.gpsimd.tensor_copy(chain(x0, 1), y01v[:, :, 1, :])

        # ---- Bs^2 kc-major with p2 = Bs@[Y0|Y1] riding each kc-group ----
        b2 = [
            psq.tile([128, D], F32, tag="sq", name=f"b2_{r}")
            for r in range(KC)
        ]
        p2 = ps_mv.tile([128, KC * 2 * BLOC], F32, tag="pmv", name="p2")
        for kc in range(KC):
            for r in range(KC):
                nc.tensor.matmul(
                    b2[r][:],
                    bs[kc][:, r * 128 : (r + 1) * 128],
                    bs[kc][:],
                    start=(kc == 0),
                    stop=(kc == KC - 1),
                )
        # p2 = Bs@[Y0|Y1], mc-major: the 4 mc accumulation groups share one
        # PSUM bank, and start=True clears has_written bank-wide, so groups
        # must not interleave.
        for mc in range(KC):
            for kc in range(KC):
                nc.tensor.matmul(
                    p2[:, mc * 2 * BLOC : (mc + 1) * 2 * BLOC],
                    bs[kc][:, mc * 128 : (mc + 1) * 128],
                    y01[:, kc * 2 * BLOC : (kc + 1) * 2 * BLOC].bitcast(F32R),
                    start=(kc == 0),
                    stop=(kc == KC - 1),
                )

        # Y2 = 0.5*Bs@Y0; Y3 = Bs@Y1 - g^2*Y1 (straight into the bf16 chain)
        p2v = p2[:].rearrange("p (k r i) -> p k r i", k=KC, r=2)
        nc.vector.tensor_scalar_mul(chain(x0, 2), p2v[:, :, 0, :], 0.5)
        nc.vector.scalar_tensor_tensor(
            chain(x0, 3), y01v[:, :, 1, :], -G2, p2v[:, :, 1, :], AL.mult, AL.add
        )

        # ---- T4h = Bs^2 - 2g^4*I, split bf16 hi+lo straight from PSUM ----
        thi = [
            const.tile([128, D], BF16, tag=f"thi{r}", name=f"thi{r}")
            for r in range(KC)
        ]
        tlo = [
            const.tile([128, D], BF16, tag=f"tlo{r}", name=f"tlo{r}")
            for r in range(KC)
        ]
        for r in range(KC):
            db = b2[r][:, r * 128 : (r + 1) * 128]
            nc.vector.scalar_tensor_tensor(db, eye, -TWO_G4, db, AL.mult, AL.add)
            if r < 2:
                nc.scalar.copy(thi[r][:], b2[r][:])
            else:
                nc.vector.tensor_copy(thi[r][:], b2[r][:])
            nc.vector.tensor_sub(tlo[r][:], b2[r][:], thi[r][:])

        def acc_step(st, s, first=False):
            """acc[:, kc, r, i] += c[4s+r, i] * st[:, kc, r, i] (chain-resolved;
            the r-reduction happens once at the very end)."""
            if first:
                nc.vector.tensor_mul(
                    acc[:].rearrange("p (k w) -> p k w", k=KC),
                    st[:].rearrange("p (k w) -> p k w", k=KC),
                    cstep(s),
                )
            else:
                mt = work.tile([128, KC * W], F32, tag="mt")
                nc.vector.tensor_mul(
                    mt[:].rearrange("p (k w) -> p k w", k=KC),
                    st[:].rearrange("p (k w) -> p k w", k=KC),
                    cstep(s),
                )
                nc.vector.tensor_add(acc[:], acc[:], mt[:])

        acc_step(x0, 0, first=True)

        def matstep(dst_a, dst_b, st):
            """halves a (mc 0,1) / b (mc 2,3) += T4h-block @ st, bf16 hi/lo
            weights.  All-hi then all-lo per dst so the hi matmuls can start
            before the lo split finishes; two bank-separate PSUM tiles so the
            consumer of half a starts while half b's matmuls run."""
            for mc in range(KC):
                dst = dst_a if mc < 2 else dst_b
                mo = mc % 2
                first = True
                for mats in (thi, tlo):
                    for kc in range(KC):
                        nc.tensor.matmul(
                            dst[:, mo * W : (mo + 1) * W],
                            mats[kc][:, mc * 128 : (mc + 1) * 128],
                            st[:, kc * W : (kc + 1) * W],
                            start=first,
                            stop=(kc == KC - 1 and mats is tlo),
                        )
                        first = False

        # ---- step 1 (special): X1[r] = T4h@X0[r] - g^{2r}*Y_{4-r}; r=0 halved
        x1 = xs[1]
        za = ps_stp.tile([128, 2 * W], F32, tag="stpA", name="z1a")
        zb = ps_stp.tile([128, 2 * W], F32, tag="stpB", name="z1b")
        matstep(za, zb, x0[:])
        x1v = x1[:].rearrange("p (k r i) -> p k r i", k=KC, r=NCH)
        x0v = x0[:].rearrange("p (k r i) -> p k r i", k=KC, r=NCH)
        for h, zt in ((0, za), (1, zb)):
            zv = zt[:].rearrange("p (k r i) -> p k r i", k=2, r=NCH)
            ks = slice(2 * h, 2 * h + 2)
            nc.vector.tensor_scalar_mul(x1v[:, ks, 0, :], zv[:, :, 0, :], 0.5)
            for r in (1, 2, 3):
                nc.vector.scalar_tensor_tensor(
                    x1v[:, ks, r, :],
                    x0v[:, ks, NCH - r, :],
                    -(GAMMA ** (2 * r)),
                    zv[:, :, r, :],
                    AL.mult,
                    AL.add,
                )
        acc_step(x1, 1)

        # ---- steps 2..NSTEP: Xn = T4h@Xc - gamma^8*Xp ----
        xp_, xc, xn = xs
        outv = outT[:].rearrange("(k p) i -> p k i", p=128)
        res = state.tile([128, KC * BLOC], F32, tag="res")
        for s in range(2, NSTEP + 1):
            Pa = ps_stp.tile([128, 2 * W], F32, tag="stpA", name=f"P{s}a")
            Pb = ps_stp.tile([128, 2 * W], F32, tag="stpB", name=f"P{s}b")
            matstep(Pa, Pb, xc[:])
            last = s == NSTEP
            for kc in range(KC):  # chunked: next step's kc=0 mm starts early
                sl = slice(kc * W, (kc + 1) * W)
                Ph = Pa if kc < 2 else Pb
                po = kc % 2
                nc.vector.scalar_tensor_tensor(
                    xn[:, sl], xp_[:, sl], -G8,
                    Ph[:, po * W : (po + 1) * W], AL.mult, AL.add
                )
                if last:
                    # finish acc for this chunk, reduce over chains, ship out
                    eng = nc.vector if kc % 2 == 0 else nc.gpsimd
                    mt = work.tile([128, W], F32, tag="mtc", name=f"mtc{kc}")
                    eng.tensor_mul(
                        mt[:], xn[:, sl], c_sb[:, s * W : (s + 1) * W]
                    )
                    eng.tensor_add(mt[:], mt[:], acc[:, sl])
                    rt = res[:, kc * BLOC : (kc + 1) * BLOC]
                    if eng is nc.vector:
                        eng.tensor_reduce(
                            rt.unsqueeze(1),
                            mt[:].rearrange("p (r i) -> p i r", r=NCH),
                            AX.X,
                            AL.add,
                        )
                    else:  # gpsimd: X-axis reduce unsupported; 3 small adds
                        h = work.tile([128, 2 * BLOC], F32, tag="mtc", name=f"h{kc}")
                        eng.tensor_add(
                            h[:, 0:BLOC], mt[:, 0:BLOC], mt[:, BLOC : 2 * BLOC]
                        )
                        eng.tensor_add(
                            h[:, BLOC:], mt[:, 2 * BLOC : 3 * BLOC], mt[:, 3 * BLOC :]
                        )
                        eng.tensor_add(rt, h[:, 0:BLOC], h[:, BLOC:])
                    if kc == 1:
                        nc.sync.dma_start(
                            outv[:, 0:2, :], res[:, 0 : 2 * BLOC].rearrange(
                                "p (k i) -> p k i", k=2
                            )
                        )
                    elif kc == 3:
                        nc.scalar.dma_start(
                            outv[:, 2:4, :], res[:, 2 * BLOC :].rearrange(
                                "p (k i) -> p k i", k=2
                            )
                        )
            if not last:
                acc_step(xn, s)
            xp_, xc, xn = xc, xn, xp_

    nc.compile()
    return nc


_NC_CACHE = {}


def _get_nc():
    if "nc" not in _NC_CACHE:
        _NC_CACHE["nc"] = _build_nc()
    return _NC_CACHE["nc"]


def build_in_maps(t, x, mean0, cov0):
    t = np.ascontiguousarray(t, np.float32)
    x = np.ascontiguousarray(x, np.float32)
    mean0 = np.ascontiguousarray(mean0, np.float32)
    cov0 = np.ascontiguousarray(cov0, np.float32)
    lam, dmat = _host_constants()
    mean_pk = np.ascontiguousarray(mean0.reshape(KC, 128).T)
    eye = np.eye(128, dtype=np.float32)
    in_maps = []
    for i in range(NCORES):
        sl = slice(i * BLOC, (i + 1) * BLOC)
        xpack = np.zeros((128, XP_W), np.float32)
        # xhat: [p, (kc, i)] of x_shard.T reshaped
        xpack[:, XP_X : XP_X + KC * BLOC] = (
            x[sl].T.reshape(KC, 128, BLOC).transpose(1, 0, 2).reshape(128, KC * BLOC)
        )
        xpack[:, XP_M : XP_M + KC] = mean_pk
        xpack[0, XP_T : XP_T + BLOC] = t[sl]
        xpack[:, XP_EYE : XP_EYE + 128] = eye
        xpack[0:NN, XP_LAM : XP_LAM + 1] = lam
        xpack[0:NN, XP_DM : XP_DM + NK + 1] = dmat
        in_maps.append({"xpack": xpack, "cov0": cov0})
    return in_maps


def gather(results):
    out = np.empty((B, D), np.float32)
    for i in range(NCORES):
        out[i * BLOC : (i + 1) * BLOC, :] = results[i]["outT"].T
    return out


def kernel(t, x, mean0, cov0):
    nc = _get_nc()
    in_maps = build_in_maps(t, x, mean0, cov0)
    res = run_bass_kernel_spmd(nc, in_maps, core_ids=list(range(NCORES)))
    return gather(res.results)


# revision 8
# speedup vs baseline: 1.0377x; 1.0377x over previous
"""Analytic Gaussian VP score on 8 TRN2 NeuronCores.

Math: per sample i, score_i = -Sigma_i^{-1} (x_i - a_i*mean0) with
Sigma_i = a_i^2*cov0 + s_i^2*I.  All Sigma_i are shifted/scaled versions of
one shared matrix, so instead of 128 per-sample Choleskys we apply a
per-sample degree-NK Chebyshev polynomial of cov0:

    score_i = -sum_k c_{i,k} T_k(Mt) u_i,   Mt = (cov0 - MID*I)/HALF

The coefficients c_{i,k} are computed ON DEVICE from t_i (ScalarE exp +
DVE + small PE matmuls).  T_k are generated four at a time via
T4h = 2*gamma^4*T_4(Mt) built from two matrix squarings; the identity
T_{k+4} = 2*T4*T_k - T_{k-4} advances four chains per batched matmul so
the TensorEngine runs [128,128]-weight matmuls with a 64-wide moving
operand.  Basis scaling gamma = HALF/2 folds all affine constants into
shifted matrices: Cs = C - MID*I (diag-block fixups only), Bs = Cs**2
- 2g^2*I, T4h = Bs**2 - 2g^4*I, with Y1 = 0.5*Cs@Y0, Y2 = 0.5*Bs@Y0,
Y3 = Bs@Y1 - g^2*Y1 - so no full-matrix elementwise passes exist
between the squarings, only [128,128] diag ops and the PSUM->SBUF
moves that are needed anyway.

Perf structure (from perfetto/NTFF analysis of the previous version):
- every HWDGE dma_start costs ~600-790ns of issue time on its engine, so
  all small inputs (x, mean, t, eye, lam, dmat) are host-packed into ONE
  [128, 229] tensor; cov0 ships as 4 chunk DMAs split over both HWDGE
  queues; the ones-vectors are gpsimd memsets (6 input DMAs, was 14).
- C^2 runs kc-major (each cov chunk's 4 matmuls fire on chunk arrival,
  accumulating into 4 PSUM banks) instead of waiting for the full 1MB.
- the PE stream (warmup -> Cs^2 -> p1 -> Bs^2(+p2 riding) -> chain
  steps) has no >3.4us gaps, so the HAM clock gate releases once at
  ~10us and stays at 2.4GHz (the old version re-throttled mid-kernel).
- NK=15 (16 coeffs = 4 chains x 4 steps): pole-model error ~9.5e-3 vs
  the 2e-2 gate (measured 2.7e-3 at NK=19, model-accurate).
- ~7us of exec window (runtime preamble + walrus' 253-semaphore clear
  epilogue + final barriers) is fixed overhead outside kernel control.

Sharding: pure data parallelism - mean0/cov0 replicated, the 128 (t, x)
pairs split 16 per core, no collectives.  State kept transposed
([feature, (chunk, chain, sample)]) so matmuls are cov0-stationary
(symmetric blocks, no transposes).  float32r on construction matmuls,
bf16 hi+lo split of T4h (~2^-16 effective weights) on chain steps.
"""

import numpy as np

try:
    import concourse.bass as bass
except ImportError:  # fresh grading dir: point at the staged repo
    import sys

    for _p in ("/opt/trn_rl_repo", "/root/.axon_site/_ro/trn_rl_repo"):
        if _p not in sys.path:
            sys.path.insert(0, _p)
    import concourse.bass as bass

from contextlib import ExitStack

import concourse.tile as tile
from concourse import bacc, mybir
from concourse.tile import ScopedClock


def _lean_drain_and_barrier(self, tick_clock, wait_clock):
    """Tile end-sequence without the TileContext per-semaphore end-clears.

    Bass clears the kernel semaphore range at NEFF entry, so for a
    single-TileContext kernel the end-of-kernel clear_and_free pass is
    redundant.  Keep the drain (fences DMA queues) and one barrier.
    """
    drain_inst = self.nc.sync.drain()
    wait_clock.add_sem_waits(
        drain_inst.ins, ScopedClock({None: tick_clock.global_clock})
    )
    self.nc.all_engine_barrier()
    popped = self.nc._tile_sem_poison_stack.pop()
    assert popped is self._sem_poison
from concourse.bass_utils import run_bass_kernel_spmd

F32 = mybir.dt.float32
F32R = mybir.dt.float32r
BF16 = mybir.dt.bfloat16
AL = mybir.AluOpType
AX = mybir.AxisListType

B, D = 128, 512
NCORES = 8
BLOC = B // NCORES  # 16 samples per core
KC = D // 128  # 4 partition chunks of the feature dim
NCH = 4  # Chebyshev chains advanced per step
W = NCH * BLOC  # 64: moving-operand width of the chain matmuls

# Chebyshev setup (input-independent constants)
L_BND, U_BND = 0.0995, 4.10
NN = 64  # interpolation nodes
NK = 15  # polynomial degree; NK+1 = 16 coefficients = 4 chains x 4 steps
NSTEP = (NK + 1) // 4 - 1  # chain steps after init (first one special)
MID = (U_BND + L_BND) / 2.0
HALF = (U_BND - L_BND) / 2.0
GAMMA = HALF / 2.0  # basis scaling; matmuls consume raw shifted matrices
G2 = GAMMA * GAMMA
G8 = GAMMA**8
TWO_G2 = 2.0 * G2  # Bs = Cs^2 - 2g^2 I
TWO_G4 = 2.0 * GAMMA**4  # T4h = Bs^2 - 2g^4 I
BETA_MIN, BETA_MAX = 0.1, 20.0

# cpack column layout (tiny, lands first): [mhat 4 | trow 16 (row 0) |
#   lam 1 (rows 0:NN) | dmat NK+1 (rows 0:NN)]; xeye = [xhat 64 | eye 128]
CP_M = 0
CP_T = CP_M + KC
CP_LAM = CP_T + BLOC
CP_DM = CP_LAM + 1
CP_W = 128  # padded so rows are 512B (SDMA line-rate minimum)
XE_X = 0
XE_EYE = XE_X + KC * BLOC
XE_W = XE_EYE + 128


def _host_constants():
    j = np.arange(NN)
    th = np.pi * (j + 0.5) / NN
    lam = (MID + HALF * np.cos(th)).astype(np.float32).reshape(NN, 1)
    k = np.arange(NK + 1)
    dm = (2.0 / NN) * np.cos(k[None, :] * th[:, None])
    dm[:, 0] *= 0.5
    dm = (-dm) * (1.0 / np.float64(GAMMA)) ** k[None, :]  # fold -1, gamma^-k
    dmat = dm.astype(np.float32)
    return lam, dmat


def _build_nc():
    nc = bacc.Bacc()
    cpack = nc.declare_dram_parameter("cpack", [128, CP_W], F32, isOutput=False)
    xeye = nc.declare_dram_parameter("xeye", [128, XE_W], F32, isOutput=False)
    cov0 = nc.declare_dram_parameter("cov0", [D, D], F32R, isOutput=False)
    outT = nc.declare_dram_parameter("outT", [D, BLOC], F32, isOutput=True)

    with ExitStack() as ctx:
        tc = ctx.enter_context(tile.TileContext(nc))
        tc._drain_and_barrier = _lean_drain_and_barrier.__get__(tc)
        const = ctx.enter_context(tc.tile_pool(name="const", bufs=1))
        state = ctx.enter_context(tc.tile_pool(name="state", bufs=1))
        work = ctx.enter_context(tc.tile_pool(name="work", bufs=2))
        psq = ctx.enter_context(tc.tile_pool(name="psq", bufs=4, space="PSUM"))
        ps_stp = ctx.enter_context(tc.tile_pool(name="ps_stp", bufs=1, space="PSUM"))
        ps_mv = ctx.enter_context(tc.tile_pool(name="ps_mv", bufs=1, space="PSUM"))
        ps_one = ctx.enter_context(tc.tile_pool(name="ps_one", bufs=1, space="PSUM"))

        # ---- PE warm-up: memset a scratch tile (no DMA dependency) + dummy
        # fp32 matmuls so the HAM clock gate sees activity from the start;
        # sized to end roughly when the first cov chunk lands (~1us).
        warm_sb = const.tile([128, 128], F32, tag="warm_sb")
        nc.gpsimd.memset(warm_sb[:], 1.0)
        warm_ps = ps_one.tile([128, (NK + 1) * BLOC], F32, tag="one", name="warm_ps")
        for _ in range(3):
            nc.tensor.matmul(warm_ps[:, 0:128], warm_sb[:], warm_sb[:])

        # ---- loads.  The whole input set is HBM-bandwidth-bound (~300 GB/s
        # pooled across both HWDGE queues; per-packet round-robin), so order
        # by need: tiny cpack first (unblocks the DVE/ACT preamble at the
        # first packets), cov chunk 0 first on sync (starts C^2), then the
        # rest.  cov0 arrives already shifted by -MID*I from the host. ----
        cs = [
            const.tile([128, D], F32R, tag=f"cs{k}", name=f"cs{k}")
            for k in range(KC)
        ]
        cp = const.tile([128, CP_W], F32, tag="cp")
        xe = const.tile([128, XE_W], F32, tag="xe")
        nc.sync.dma_start(cs[0][:], cov0[0:128, :])
        nc.scalar.dma_start(cp[:], cpack[:])
        nc.scalar.dma_start(xe[:], xeye[:])
        nc.sync.dma_start(cs[1][:], cov0[128:256, :])
        nc.scalar.dma_start(cs[2][:], cov0[256:384, :])
        nc.sync.dma_start(cs[3][:], cov0[384:512, :])

        eye = xe[:, XE_EYE : XE_EYE + 128]
        xhat = xe[:, XE_X : XE_X + KC * BLOC]
        mhat = cp[:, CP_M : CP_M + KC]
        trow = cp[0:1, CP_T : CP_T + BLOC]
        lam_sb = cp[0:NN, CP_LAM : CP_LAM + 1]
        dmat_sb = cp[0:NN, CP_DM : CP_DM + NK + 1]

        ones1 = const.tile([1, 128], F32, tag="ones1")
        nc.gpsimd.memset(ones1[:], 1.0)
        ones_nn = const.tile([NN, 128], F32, tag="ones_nn")
        nc.gpsimd.memset(ones_nn[:], 1.0)

        # ---- Cs^2 kc-major: each chunk's 4 matmuls fire on arrival ----
        c2 = [
            psq.tile([128, D], F32, tag="sq", name=f"c2_{r}")
            for r in range(KC)
        ]
        for kc in range(KC):
            for r in range(KC):
                nc.tensor.matmul(
                    c2[r][:],
                    cs[kc][:, r * 128 : (r + 1) * 128],
                    cs[kc][:],
                    start=(kc == 0),
                    stop=(kc == KC - 1),
                )

        # ---- per-sample scalars from t (overlaps the cov DMAs) ----
        u9 = const.tile([1, BLOC], F32, tag="u9")
        nc.vector.tensor_scalar(u9[:], trow, 9.95, 0.1, AL.mult, AL.add)
        ib = const.tile([1, BLOC], F32, tag="ib")
        nc.vector.tensor_mul(ib[:], u9[:], trow)
        a_row = const.tile([1, BLOC], F32, tag="a_row")
        nc.scalar.activation(
            a_row[:], ib[:], mybir.ActivationFunctionType.Exp, scale=-0.5
        )
        abc = const.tile([1, 3 * BLOC], F32, tag="abc")  # [a | a^2 | s^2]
        nc.vector.tensor_copy(abc[:, 0:BLOC], a_row[:])
        nc.vector.tensor_mul(abc[:, BLOC : 2 * BLOC], a_row[:], a_row[:])
        nc.vector.tensor_scalar(
            abc[:, 2 * BLOC :], abc[:, BLOC : 2 * BLOC], -1.0, 1.0, AL.mult, AL.add
        )
        nc.vector.tensor_scalar_max(abc[:, 2 * BLOC :], abc[:, 2 * BLOC :], 1e-12)

        # broadcast [a | a^2 | s^2] down all 128 partitions via a K=1 matmul
        rep_ps = ps_one.tile([128, (NK + 1) * BLOC], F32, tag="one", name="rep_ps")
        nc.tensor.matmul(rep_ps[:, 0 : 3 * BLOC], ones1[:], abc[:])
        rep = const.tile([128, 3 * BLOC], F32, tag="rep_sb")
        nc.scalar.copy(rep[:], rep_ps[:, 0 : 3 * BLOC])
        a_rep = rep[:, 0:BLOC]
        a2_rep = rep[:, BLOC : 2 * BLOC]
        s2_rep = rep[:, 2 * BLOC : 3 * BLOC]

        # ---- Chebyshev coefficients on device ----
        q = const.tile([NN, BLOC], F32, tag="q")
        nc.vector.scalar_tensor_tensor(
            q[:], a2_rep[0:NN, :], lam_sb[:, 0:1], s2_rep[0:NN, :], AL.mult, AL.add
        )
        fhat = const.tile([NN, BLOC], F32, tag="fhat")
        nc.vector.reciprocal(fhat[:], q[:])
        rhs_t = const.tile([NN, (NK + 1) * BLOC], F32, tag="rhs_t")
        nc.vector.tensor_mul(
            rhs_t[:].rearrange("p (k i) -> p k i", k=NK + 1),
            fhat[:].unsqueeze(1).broadcast_to((NN, NK + 1, BLOC)),
            dmat_sb.unsqueeze(2).broadcast_to((NN, NK + 1, BLOC)),
        )
        c_ps = ps_one.tile([128, (NK + 1) * BLOC], F32, tag="one", name="c_ps")
        c_sb = const.tile([128, (NK + 1) * BLOC], F32, tag="c_sb")

        def cstep(s):
            """coefficients for step s: [128, (chain, sample)] bcast over kc."""
            return (
                c_sb[:, s * W : (s + 1) * W]
                .unsqueeze(1)
                .broadcast_to((128, KC, W))
            )

        # ---- state: X [128, (kc, chain, sample)], bf16 ----
        xs = [
            state.tile([128, KC * W], BF16, tag=f"X{i}", name=f"X{i}")
            for i in range(3)
        ]
        y01 = state.tile([128, KC * 2 * BLOC], F32R, tag="y01")  # f32 Y0,Y1
        acc = state.tile([128, KC * W], F32, tag="acc")

        def chain(st, r):
            """[128, kc, BLOC] view of chain r of state tile st."""
            return st[:].rearrange("p (k r i) -> p k r i", k=KC, r=NCH)[:, :, r, :]

        def v3(ap):
            return ap.rearrange("p (k i) -> p k i", k=KC)

        # ---- init: Y0 = u = x - a*mean0 (f32 scratch y01 + bf16 chain) ----
        x0 = xs[0]
        y01v = y01[:].rearrange("p (k r i) -> p k r i", k=KC, r=2)
        w1 = work.tile([128, KC * BLOC], F32, tag="w1")
        nc.vector.tensor_mul(
            v3(w1[:]),
            a_rep.unsqueeze(1).broadcast_to((128, KC, BLOC)),
            mhat.unsqueeze(2).broadcast_to((128, KC, BLOC)),
        )
        nc.vector.tensor_sub(y01v[:, :, 0, :], v3(xhat), v3(w1[:]))
        nc.gpsimd.tensor_copy(chain(x0, 0), y01v[:, :, 0, :])

        # ---- Bs = Cs^2 - 2g^2*I in SBUF fp32r (PSUM->SBUF moves split
        # between ScalarE and DVE; diag fixed in place) ----
        bs = [
            const.tile([128, D], F32R, tag=f"bs{r}", name=f"bs{r}")
            for r in range(KC)
        ]
        for r in range(KC):
            if r < 2:
                nc.scalar.copy(bs[r][:], c2[r][:])
            else:
                nc.vector.tensor_copy(bs[r][:], c2[r][:])
            db = bs[r][:, r * 128 : (r + 1) * 128]
            nc.vector.scalar_tensor_tensor(db, eye, -TWO_G2, db, AL.mult, AL.add)

        nc.tensor.matmul(c_ps[:], ones_nn[:], rhs_t[:])
        nc.scalar.copy(c_sb[:], c_ps[:])

        # ---- p1: Y1 = 0.5*Cs@Y0 (runs while the Bs copies drain) ----
        p1 = ps_mv.tile([128, KC * BLOC], F32, tag="pmv", name="p1")
        for mc in range(KC):
            for kc in range(KC):
                nc.tensor.matmul(
                    p1[:, mc * BLOC : (mc + 1) * BLOC],
                    cs[kc][:, mc * 128 : (mc + 1) * 128],
                    y01v[:, kc, 0, :].bitcast(F32R),
                    start=(kc == 0),
                    stop=(kc == KC - 1),
                )
        nc.vector.tensor_scalar_mul(y01v[:, :, 1, :], v3(p1[:]), 0.5)
        nc.gpsimd.tensor_copy(chain(x0, 1), y01v[:, :, 1, :])

        # ---- Bs^2 kc-major with p2 = Bs@[Y0|Y1] riding each kc-group ----
        b2 = [
            psq.tile([128, D], F32, tag="sq", name=f"b2_{r}")
            for r in range(KC)
        ]
        p2 = ps_mv.tile([128, KC * 2 * BLOC], F32, tag="pmv", name="p2")
        for kc in range(KC):
            for r in range(KC):
                nc.tensor.matmul(
                    b2[r][:],
                    bs[kc][:, r * 128 : (r + 1) * 128],
                    bs[kc][:],
                    start=(kc == 0),
                    stop=(kc == KC - 1),
                )
        # p2 = Bs@[Y0|Y1], mc-major: the 4 mc accumulation groups share one
        # PSUM bank, and start=True clears has_written bank-wide, so groups
        # must not interleave.
        for mc in range(KC):
            for kc in range(KC):
                nc.tensor.matmul(
                    p2[:, mc * 2 * BLOC : (mc + 1) * 2 * BLOC],
                    bs[kc][:, mc * 128 : (mc + 1) * 128],
                    y01[:, kc * 2 * BLOC : (kc + 1) * 2 * BLOC].bitcast(F32R),
                    start=(kc == 0),
                    stop=(kc == KC - 1),
                )

        # Y2 = 0.5*Bs@Y0; Y3 = Bs@Y1 - g^2*Y1 (straight into the bf16 chain)
        p2v = p2[:].rearrange("p (k r i) -> p k r i", k=KC, r=2)
        nc.vector.tensor_scalar_mul(chain(x0, 2), p2v[:, :, 0, :], 0.5)
        nc.vector.scalar_tensor_tensor(
            chain(x0, 3), y01v[:, :, 1, :], -G2, p2v[:, :, 1, :], AL.mult, AL.add
        )

        # ---- T4h = Bs^2 - 2g^4*I, split bf16 hi+lo straight from PSUM ----
        thi = [
            const.tile([128, D], BF16, tag=f"thi{r}", name=f"thi{r}")
            for r in range(KC)
        ]
        tlo = [
            const.tile([128, D], BF16, tag=f"tlo{r}", name=f"tlo{r}")
            for r in range(KC)
        ]
        for r in range(KC):
            db = b2[r][:, r * 128 : (r + 1) * 128]
            nc.vector.scalar_tensor_tensor(db, eye, -TWO_G4, db, AL.mult, AL.add)
            if r < 2:
                nc.scalar.copy(thi[r][:], b2[r][:])
            else:
                nc.vector.tensor_copy(thi[r][:], b2[r][:])
            nc.vector.tensor_sub(tlo[r][:], b2[r][:], thi[r][:])

        def acc_step(st, s, first=False):
            """acc[:, kc, r, i] += c[4s+r, i] * st[:, kc, r, i] (chain-resolved;
            the r-reduction happens once at the very end)."""
            if first:
                nc.vector.tensor_mul(
                    acc[:].rearrange("p (k w) -> p k w", k=KC),
                    st[:].rearrange("p (k w) -> p k w", k=KC),
                    cstep(s),
                )
            else:
                mt = work.tile([128, KC * W], F32, tag="mt")
                nc.vector.tensor_mul(
                    mt[:].rearrange("p (k w) -> p k w", k=KC),
                    st[:].rearrange("p (k w) -> p k w", k=KC),
                    cstep(s),
                )
                nc.vector.tensor_add(acc[:], acc[:], mt[:])

        acc_step(x0, 0, first=True)

        def matstep(dst_a, dst_b, st):
            """halves a (mc 0,1) / b (mc 2,3) += T4h-block @ st, bf16 hi/lo
            weights.  All-hi then all-lo per dst so the hi matmuls can start
            before the lo split finishes; two bank-separate PSUM tiles so the
            consumer of half a starts while half b's matmuls run."""
            for mc in range(KC):
                dst = dst_a if mc < 2 else dst_b
                mo = mc % 2
                first = True
                for mats in (thi, tlo):
                    for kc in range(KC):
                        nc.tensor.matmul(
                            dst[:, mo * W : (mo + 1) * W],
                            mats[kc][:, mc * 128 : (mc + 1) * 128],
                            st[:, kc * W : (kc + 1) * W],
                            start=first,
                            stop=(kc == KC - 1 and mats is tlo),
                        )
                        first = False

        # ---- step 1 (special): X1[r] = T4h@X0[r] - g^{2r}*Y_{4-r}; r=0 halved
        x1 = xs[1]
        za = ps_stp.tile([128, 2 * W], F32, tag="stpA", name="z1a")
        zb = ps_stp.tile([128, 2 * W], F32, tag="stpB", name="z1b")
        matstep(za, zb, x0[:])
        x1v = x1[:].rearrange("p (k r i) -> p k r i", k=KC, r=NCH)
        x0v = x0[:].rearrange("p (k r i) -> p k r i", k=KC, r=NCH)
        for h, zt in ((0, za), (1, zb)):
            zv = zt[:].rearrange("p (k r i) -> p k r i", k=2, r=NCH)
            ks = slice(2 * h, 2 * h + 2)
            nc.vector.tensor_scalar_mul(x1v[:, ks, 0, :], zv[:, :, 0, :], 0.5)
            for r in (1, 2, 3):
                nc.vector.scalar_tensor_tensor(
                    x1v[:, ks, r, :],
                    x0v[:, ks, NCH - r, :],
                    -(GAMMA ** (2 * r)),
                    zv[:, :, r, :],
                    AL.mult,
                    AL.add,
                )
        acc_step(x1, 1)

        # ---- steps 2..NSTEP: Xn = T4h@Xc - gamma^8*Xp ----
        xp_, xc, xn = xs
        outv = outT[:].rearrange("(k p) i -> p k i", p=128)
        res = state.tile([128, KC * BLOC], F32, tag="res")
        for s in range(2, NSTEP + 1):
            Pa = ps_stp.tile([128, 2 * W], F32, tag="stpA", name=f"P{s}a")
            Pb = ps_stp.tile([128, 2 * W], F32, tag="stpB", name=f"P{s}b")
            matstep(Pa, Pb, xc[:])
            last = s == NSTEP
            for kc in range(KC):  # chunked: next step's kc=0 mm starts early
                sl = slice(kc * W, (kc + 1) * W)
                Ph = Pa if kc < 2 else Pb
                po = kc % 2
                nc.vector.scalar_tensor_tensor(
                    xn[:, sl], xp_[:, sl], -G8,
                    Ph[:, po * W : (po + 1) * W], AL.mult, AL.add
                )
                if last:
                    # finish acc for this chunk, reduce over chains, ship out
                    mt = work.tile([128, W], F32, tag="mtc", name=f"mtc{kc}")
                    nc.vector.tensor_mul(
                        mt[:], xn[:, sl], c_sb[:, s * W : (s + 1) * W]
                    )
                    nc.vector.tensor_add(mt[:], mt[:], acc[:, sl])
                    rt = res[:, kc * BLOC : (kc + 1) * BLOC]
                    nc.vector.tensor_reduce(
                        rt.unsqueeze(1),
                        mt[:].rearrange("p (r i) -> p i r", r=NCH),
                        AX.X,
                        AL.add,
                    )
                    if kc == 1:
                        nc.sync.dma_start(
                            outv[:, 0:2, :], res[:, 0 : 2 * BLOC].rearrange(
                                "p (k i) -> p k i", k=2
                            )
                        )
                    elif kc == 3:
                        nc.scalar.dma_start(
                            outv[:, 2:4, :], res[:, 2 * BLOC :].rearrange(
                                "p (k i) -> p k i", k=2
                            )
                        )
            if not last:
                acc_step(xn, s)
            xp_, xc, xn = xc, xn, xp_

    nc.compile()
    return nc


_NC_CACHE = {}


def _get_nc():
    if "nc" not in _NC_CACHE:
        _NC_CACHE["nc"] = _build_nc()
    return _NC_CACHE["nc"]


def build_in_maps(t, x, mean0, cov0):
    t = np.ascontiguousarray(t, np.float32)
    x = np.ascontiguousarray(x, np.float32)
    mean0 = np.ascontiguousarray(mean0, np.float32)
    cov0 = np.ascontiguousarray(cov0, np.float32)
    cov0s = cov0 - MID * np.eye(D, dtype=np.float32)  # Cs = C - MID*I
    lam, dmat = _host_constants()
    mean_pk = np.ascontiguousarray(mean0.reshape(KC, 128).T)
    eye = np.eye(128, dtype=np.float32)
    cpack0 = np.zeros((128, CP_W), np.float32)
    cpack0[:, CP_M : CP_M + KC] = mean_pk
    cpack0[0:NN, CP_LAM : CP_LAM + 1] = lam
    cpack0[0:NN, CP_DM : CP_DM + NK + 1] = dmat
    in_maps = []
    for i in range(NCORES):
        sl = slice(i * BLOC, (i + 1) * BLOC)
        cpack = cpack0.copy()
        cpack[0, CP_T : CP_T + BLOC] = t[sl]
        xeye = np.zeros((128, XE_W), np.float32)
        # xhat: [p, (kc, i)] of x_shard.T reshaped
        xeye[:, XE_X : XE_X + KC * BLOC] = (
            x[sl].T.reshape(KC, 128, BLOC).transpose(1, 0, 2).reshape(128, KC * BLOC)
        )
        xeye[:, XE_EYE : XE_EYE + 128] = eye
        in_maps.append({"cpack": cpack, "xeye": xeye, "cov0": cov0s})
    return in_maps


def gather(results):
    out = np.empty((B, D), np.float32)
    for i in range(NCORES):
        out[i * BLOC : (i + 1) * BLOC, :] = results[i]["outT"].T
    return out


def kernel(t, x, mean0, cov0):
    nc = _get_nc()
    in_maps = build_in_maps(t, x, mean0, cov0)
    res = run_bass_kernel_spmd(nc, in_maps, core_ids=list(range(NCORES)))
    return gather(res.results)
